# revision 1
# baseline (speedup 1.0000x reference)
"""Trainium2 Bass kernel for ContactDiffusion GNN message passing.

out = latent + K_norm @ msg,  K = (D+eps)^(-alpha_ij) * exp(-D/12), row-normalized,
msg = MLP(latent).

Strategy (8 NeuronCores, SPMD, full inputs in / full output out):
 - Host: KD-sort points spatially; each core owns 1024 contiguous sorted rows.
 - Device per core: pairwise d2 for its [8192 x 1024] K^T slab via a Gram-form
   fp16-split feature matmul (k=18), elementwise K chain on ScalarE
   (ln / exp, single activation-table set), contraction + row-sums on PE.
 - The core's own diagonal block is computed exactly (ACT Square with
   per-partition bias = direct (ci-cj)^2) with exact ln(D+eps); the Gram pass
   suppresses that block via a rank-1 indicator feature.
 - Cross-core close pairs ("stragglers", d2 < 0.09) are deterministically
   suppressed on device via a second rank-1 indicator feature and their exact
   contribution is added back on host using the exported row sums.
 - MLP is sharded (each core computes msg for its rows); msg is AllGathered.
"""

import math
import os
import sys
from contextlib import ExitStack

import numpy as np

sys.path.insert(0, "/opt/trn_rl_repo")

import ml_dtypes

import concourse.bass as bass
import concourse.tile as tile
from concourse import bacc, mybir
from concourse.bass_utils import run_bass_kernel_spmd

F32 = mybir.dt.float32
F16 = mybir.dt.float16
BF16 = mybir.dt.bfloat16
AF = mybir.ActivationFunctionType
ALU = mybir.AluOpType

NP_BF16 = ml_dtypes.bfloat16

N, DIM, NCORE = 8192, 512, 8
NSH = N // NCORE            # rows per core
EPS, LAM = 1e-4, 12.0
TSTRAG = 0.09               # d2 below this across cores -> straggler
SUP = 1e3                   # suppressor feature magnitude (SUP^2 added to d2)
GROUP = 16                  # j-tiles per psum_out accumulation group
LN12 = math.log(12.0)

_BUILT = {}


# ----------------------------------------------------------------------------
# device program
# ----------------------------------------------------------------------------
def build_program(n=N, dim=DIM, nsh=NSH, group=GROUP, trace_sim=False, gelu=True, taps=False):
    nt_own = nsh // 128          # own-block j-tiles
    nt_main = n // 128           # main-pass j-tiles
    n_kd = dim // 128            # contraction k-blocks for MLP
    n_ic = nsh // 128            # i-chunks
    nt_all = nt_own + nt_main

    nc = bacc.Bacc("TRN2", target_bir_lowering=False, debug=False,
                   num_devices=NCORE)

    # ---- dram params ----
    featj = nc.dram_tensor("featj", [18, n], F16, kind="ExternalInput").ap()
    feati = nc.dram_tensor("feati", [18, nsh], F16, kind="ExternalInput").ap()
    ahj = nc.dram_tensor("ahj", [128, nt_main], F32, kind="ExternalInput").ap()
    ahjo = nc.dram_tensor("ahjo", [128, nt_own], F32, kind="ExternalInput").ap()
    ahibc = nc.dram_tensor("ahibc", [128, nsh], F32, kind="ExternalInput").ap()
    cib = nc.dram_tensor("cib", [128, 3 * nsh], F32, kind="ExternalInput").ap()
    ncjo = nc.dram_tensor("ncjo", [128, 3 * nt_own], F32, kind="ExternalInput").ap()
    latT = nc.dram_tensor("latT", [dim, nsh], F16, kind="ExternalInput").ap()
    w1t = nc.dram_tensor("w1t", [dim, dim], F16, kind="ExternalInput").ap()
    w2t = nc.dram_tensor("w2t", [dim, dim], F16, kind="ExternalInput").ap()
    b1c = nc.dram_tensor("b1c", [128, n_kd], F32, kind="ExternalInput").ap()
    b2r = nc.dram_tensor("b2r", [1, dim], F16, kind="ExternalInput").ap()
    onescol = nc.dram_tensor("onescol", [1, 128], F16, kind="ExternalInput").ap()
    ones128 = nc.dram_tensor("ones128", [128, 1], BF16, kind="ExternalInput").ap()

    num_out = nc.dram_tensor("num", [nsh, dim], F32, kind="ExternalOutput").ap()
    tap_aps = {}
    if taps:
        for tn in ["tap_d2", "tap_l", "tap_d12", "tap_t", "tap_k", "tap_kown"]:
            tap_aps[tn] = nc.dram_tensor(tn, [128, nsh], F32, kind="ExternalOutput").ap()
    srow_out = nc.dram_tensor("srow", [128, n_ic], F32, kind="ExternalOutput").ap()

    with tile.TileContext(nc, trace_sim=trace_sim) as tc, ExitStack() as ctx:
        pers = ctx.enter_context(tc.tile_pool(name="pers", bufs=1))
        p_big = ctx.enter_context(tc.tile_pool(name="pbig", bufs=2, space="PSUM"))
        p_out = ctx.enter_context(tc.tile_pool(name="pout", bufs=2, space="PSUM"))
        p_s = ctx.enter_context(tc.tile_pool(name="ps", bufs=1, space="PSUM"))
        sq_pool = ctx.enter_context(tc.tile_pool(name="sq", bufs=1))
        l_pool = ctx.enter_context(tc.tile_pool(name="lp", bufs=2))
        d12_pool = ctx.enter_context(tc.tile_pool(name="d12", bufs=2))
        amt_pool = ctx.enter_context(tc.tile_pool(name="amt", bufs=2))
        k_pool = ctx.enter_context(tc.tile_pool(name="kp", bufs=group + 4))
        kraw_pool = ctx.enter_context(tc.tile_pool(name="kraw", bufs=2))
        msg_pool = ctx.enter_context(tc.tile_pool(name="msgp", bufs=group + 4))
        dram = ctx.enter_context(tc.tile_pool(name="dram", bufs=1, space="DRAM"))
        tapp = ctx.enter_context(tc.tile_pool(name="tapp", bufs=2)) if taps else None

        dma = nc.sync.dma_start

        # ---- persistent SBUF loads ----
        featj_sb = pers.tile([18, n], F16)
        dma(featj_sb[:], featj[:])
        feati_sb = pers.tile([18, nsh], F16)
        dma(feati_sb[:], feati[:])
        ahj_sb = pers.tile([128, nt_main], F32)
        dma(ahj_sb[:], ahj[:])
        ahjo_sb = pers.tile([128, nt_own], F32)
        dma(ahjo_sb[:], ahjo[:])
        ahibc_sb = pers.tile([128, nsh], F32)
        dma(ahibc_sb[:], ahibc[:])
        cib_sb = pers.tile([128, 3 * nsh], F32)
        dma(cib_sb[:], cib[:])
        ncjo_sb = pers.tile([128, 3 * nt_own], F32)
        dma(ncjo_sb[:], ncjo[:])
        b1c_sb = pers.tile([128, n_kd], F32)
        dma(b1c_sb[:], b1c[:])
        b2r_sb = pers.tile([1, dim], F16)
        dma(b2r_sb[:], b2r[:])
        onescol_sb = pers.tile([1, 128], F16)
        dma(onescol_sb[:], onescol[:])
        ones128_sb = pers.tile([128, 1], BF16)
        dma(ones128_sb[:], ones128[:])
        latT_sb = [pers.tile([128, nsh], F16, tag=f"latT{k}", name=f"latT{k}") for k in range(n_kd)]
        for k in range(n_kd):
            dma(latT_sb[k][:], latT[k * 128:(k + 1) * 128, :])
        w1t_sb = [pers.tile([128, dim], F16, tag=f"w1t{k}", name=f"w1t{k}") for k in range(n_kd)]
        w2t_sb = [pers.tile([128, dim], F16, tag=f"w2t{k}", name=f"w2t{k}") for k in range(n_kd)]
        for k in range(n_kd):
            dma(w1t_sb[k][:], w1t[k * 128:(k + 1) * 128, :])
            dma(w2t_sb[k][:], w2t[k * 128:(k + 1) * 128, :])

        acc = pers.tile([128, n_ic * dim], F32)       # out accumulators
        nc.vector.memset(acc[:], 0.0)

        bias_ln12 = pers.tile([128, 1], F32)
        nc.gpsimd.memset(bias_ln12[:], -LN12)
        bias_eps = pers.tile([128, 1], F32)
        nc.gpsimd.memset(bias_eps[:], EPS)
        bias_ln6 = pers.tile([128, 1], F32)
        nc.gpsimd.memset(bias_ln6[:], -math.log(6.0))

        msgown_d = dram.tile([nsh, dim], BF16)
        msgall_d = dram.tile([n, dim], BF16)

        # ---- phase A: MLP (gelu table set) ----
        cw = min(512, nsh)
        hT_sb = [pers.tile([128, nsh], F16, tag=f"hT{k}", name=f"hT{k}") for k in range(n_kd)]
        for mc in range(n_kd):
            ph = p_big.tile([128, nsh], F32, tag="big", name="ph")
            for half in range(nsh // cw):
                hs = slice(half * cw, (half + 1) * cw)
                for kb in range(n_kd):
                    nc.tensor.matmul(
                        ph[:, hs],
                        lhsT=w1t_sb[kb][:, mc * 128:(mc + 1) * 128],
                        rhs=latT_sb[kb][:, hs],
                        start=(kb == 0), stop=(kb == n_kd - 1))
            nc.scalar.activation(hT_sb[mc][:], ph[:], AF.Gelu if gelu else AF.Identity,
                                 bias=b1c_sb[:, mc:mc + 1], scale=1.0)

        msgown_sb = [pers.tile([128, dim], BF16, tag=f"mo{ic}", name=f"mo{ic}") for ic in range(n_ic)]
        for ic in range(n_ic):
            pm = p_out.tile([128, dim], F32, tag="out", name="pm")
            for kb in range(n_kd):
                nc.tensor.matmul(
                    pm[:],
                    lhsT=hT_sb[kb][:, ic * 128:(ic + 1) * 128],
                    rhs=w2t_sb[kb][:],
                    start=(kb == 0), stop=False)
            nc.tensor.matmul(pm[:], lhsT=onescol_sb[:], rhs=b2r_sb[:],
                             start=False, stop=True)
            nc.scalar.copy(msgown_sb[ic][:], pm[:])
            dma(msgown_d[ic * 128:(ic + 1) * 128, :], msgown_sb[ic][:])

        # ---- phase B: AllGather msg ----
        nc.gpsimd.collective_compute(
            "AllGather", ALU.bypass,
            ins=[msgown_d.opt()], outs=[msgall_d.opt()],
            replica_groups=[list(range(NCORE))])

        # ---- phase C/D: slab loop ----
        ps_s = p_s.tile([128, n_ic], F32)

        def emit_elementwise(jt):
            """produce K tile [128, nsh] bf16 + its msg rhs tile; return both"""
            if jt < nt_own:
                # own-block exact pass
                t = jt
                sqs = []
                for d in range(3):
                    sq = sq_pool.tile([128, nsh], F32, tag=f"sq{d}")
                    nc.scalar.activation(
                        sq[:], cib_sb[:, d * nsh:(d + 1) * nsh], AF.Square,
                        bias=ncjo_sb[:, (t * 3 + d):(t * 3 + d) + 1], scale=1.0)
                    sqs.append(sq)
                nc.vector.tensor_tensor(sqs[0][:], sqs[0][:], sqs[1][:], op=ALU.add)
                nc.vector.tensor_tensor(sqs[0][:], sqs[0][:], sqs[2][:], op=ALU.add)
                l = l_pool.tile([128, nsh], F32)
                nc.scalar.activation(l[:], sqs[0][:], AF.Ln)
                d12 = d12_pool.tile([128, nsh], F32)
                nc.scalar.activation(d12[:], l[:], AF.Exp, bias=bias_ln12[:, 0:1], scale=0.5)
                bigL = amt_pool.tile([128, nsh], F32, tag="bigL")
                nc.scalar.activation(bigL[:], d12[:], AF.Ln, bias=bias_eps[:, 0:1], scale=12.0)
                al = amt_pool.tile([128, nsh], F32, tag="alpha")
                nc.vector.tensor_scalar_add(al[:], ahibc_sb[:], ahjo_sb[:, t:t + 1])
                m = amt_pool.tile([128, nsh], F32, tag="m")
                nc.vector.tensor_tensor(m[:], al[:], bigL[:], op=ALU.mult)
                tt = amt_pool.tile([128, nsh], F32, tag="t")
                nc.gpsimd.tensor_tensor(tt[:], m[:], d12[:], op=ALU.add)
                kraw = kraw_pool.tile([128, nsh], BF16)
                nc.scalar.activation(kraw[:], tt[:], AF.Exp, scale=-1.0)
                ktile = k_pool.tile([128, nsh], BF16)
                nc.gpsimd.affine_select(
                    ktile[:], kraw[:], pattern=[[1, nsh]],
                    compare_op=ALU.not_equal, fill=0.0,
                    base=-(t * 128), channel_multiplier=-1)
                if taps and t == 0:
                    tapk = tapp.tile([128, nsh], F32, tag="tap", name="tapkown")
                    nc.scalar.copy(tapk[:], ktile[:])
                    dma(tap_aps["tap_kown"][:], tapk[:])
                return ktile, msgown_sb[t]
            # main pass (gram)
            t = jt - nt_own
            pd2 = p_big.tile([128, nsh], F32, tag="big", name="pd2")
            for half in range(nsh // cw):
                hs = slice(half * cw, (half + 1) * cw)
                nc.tensor.matmul(pd2[:, hs],
                                 lhsT=featj_sb[:, t * 128:(t + 1) * 128],
                                 rhs=feati_sb[:, hs],
                                 start=True, stop=True)
            l = l_pool.tile([128, nsh], F32)
            nc.scalar.activation(l[:], pd2[:], AF.Ln)
            d12 = d12_pool.tile([128, nsh], F32)
            nc.scalar.activation(d12[:], l[:], AF.Exp, bias=bias_ln6[:, 0:1], scale=0.5)
            al = amt_pool.tile([128, nsh], F32, tag="alpha")
            nc.vector.tensor_scalar_add(al[:], ahibc_sb[:], ahj_sb[:, t:t + 1])
            m = amt_pool.tile([128, nsh], F32, tag="m")
            nc.vector.tensor_tensor(m[:], al[:], l[:], op=ALU.mult)
            tt = amt_pool.tile([128, nsh], F32, tag="t")
            nc.gpsimd.tensor_tensor(tt[:], m[:], d12[:], op=ALU.add)
            ktile = k_pool.tile([128, nsh], BF16)
            nc.scalar.activation(ktile[:], tt[:], AF.Exp, scale=-0.5)
            if taps and t == 8:
                for nm, src in [("tap_d2", pd2), ("tap_l", l), ("tap_d12", d12), ("tap_t", tt)]:
                    tp = tapp.tile([128, nsh], F32, tag="tap", name=f"tp{nm}")
                    nc.scalar.copy(tp[:], src[:])
                    dma(tap_aps[nm][:], tp[:])
                tpk = tapp.tile([128, nsh], F32, tag="tap", name="tpk")
                nc.scalar.copy(tpk[:], ktile[:])
                dma(tap_aps["tap_k"][:], tpk[:])
            mt = msg_pool.tile([128, dim], BF16)
            dma(mt[:], msgall_d[t * 128:(t + 1) * 128, :])
            return ktile, mt

        jt = 0
        while jt < nt_all:
            g = min(group, nt_all - jt)
            tiles = [emit_elementwise(jt + i) for i in range(g)]
            # row-sum matmuls (persistent psum_s accumulation)
            for i, (kt, _) in enumerate(tiles):
                for ic in range(n_ic):
                    nc.tensor.matmul(
                        ps_s[:, ic:ic + 1],
                        lhsT=kt[:, ic * 128:(ic + 1) * 128],
                        rhs=ones128_sb[:],
                        start=(jt + i == 0 and ic == 0),
                        stop=(jt + i == nt_all - 1))
            # contraction for this group
            for ic in range(n_ic):
                po = p_out.tile([128, dim], F32, tag="out", name="po")
                for i, (kt, mt) in enumerate(tiles):
                    nc.tensor.matmul(
                        po[:],
                        lhsT=kt[:, ic * 128:(ic + 1) * 128],
                        rhs=mt[:],
                        start=(i == 0), stop=(i == g - 1))
                asl = slice(ic * dim, (ic + 1) * dim)
                nc.vector.tensor_tensor(acc[:, asl], acc[:, asl], po[:], op=ALU.add)
            jt += g

        # ---- epilogue ----
        ssb = pers.tile([128, n_ic], F32)
        nc.scalar.copy(ssb[:], ps_s[:])
        dma(srow_out[:], ssb[:])
        for ic in range(n_ic):
            dma(num_out[ic * 128:(ic + 1) * 128, :],
                acc[:, ic * dim:(ic + 1) * dim])

    nc.compile()
    return nc


# ----------------------------------------------------------------------------
# host-side preprocessing
# ----------------------------------------------------------------------------
def _kdsort(coords, nblocks):
    def rec(idx, nb):
        if nb == 1:
            return [idx]
        pts = coords[idx]
        ax = int(np.argmax(pts.max(0) - pts.min(0)))
        order = np.argsort(pts[:, ax], kind="stable")
        half = len(idx) // 2
        return rec(idx[order[:half]], nb // 2) + rec(idx[order[half:]], nb // 2)

    return np.concatenate(rec(np.arange(coords.shape[0]), nblocks))


def _split16(x):
    hi = x.astype(np.float16).astype(np.float32)
    lo = (x - hi).astype(np.float16).astype(np.float32)
    return hi, lo


_erf = np.vectorize(math.erf)


def kernel(latent, coords, alpha, W1, b1, W2, b2):
    latent = np.asarray(latent, np.float32)
    coords = np.asarray(coords, np.float32)
    alpha = np.asarray(alpha, np.float32)
    W1 = np.asarray(W1, np.float32)
    b1 = np.asarray(b1, np.float32)
    W2 = np.asarray(W2, np.float32)
    b2 = np.asarray(b2, np.float32)

    perm = _kdsort(coords.astype(np.float64), 64)
    cs = coords[perm]
    als = alpha[perm]
    lats = latent[perm]
    c64 = cs.astype(np.float64)

    core_of = np.arange(N) // NSH
    # stragglers: cross-core pairs with d2 < TSTRAG
    Jstar = [set() for _ in range(NCORE)]
    Istar = [set() for _ in range(NCORE)]
    for i0 in range(0, N, 1024):
        blk = cs[i0:i0 + 1024].astype(np.float64)
        d2b = ((blk[:, None, :] - c64[None, :, :]) ** 2).sum(-1)
        d2b[np.arange(1024), np.arange(i0, i0 + 1024)] = np.inf
        ii, jj = np.nonzero(d2b < TSTRAG)
        ii = ii + i0
        msk = core_of[ii] != core_of[jj]
        for a, b in zip(ii[msk], jj[msk]):
            c = core_of[a]
            Jstar[c].add(int(b))
            Istar[c].add(int(a - c * NSH))

    r = (c64 ** 2).sum(-1).astype(np.float32)
    a2 = (-2.0 * cs).astype(np.float32)
    chj = [_split16(cs[:, d]) for d in range(3)]
    ahi = [_split16(a2[:, d]) for d in range(3)]
    rj = _split16(r)

    in_maps = []
    for core in range(NCORE):
        blk = slice(core * NSH, (core + 1) * NSH)
        rows_j, rows_i = [], []
        for d in range(3):
            for (jp, ip) in [(chj[d][0], ahi[d][0]), (chj[d][0], ahi[d][1]),
                             (chj[d][1], ahi[d][0]), (chj[d][1], ahi[d][1])]:
                rows_j.append(jp)
                rows_i.append(ip[blk])
        ones = np.ones(N, np.float32)
        onesi = np.ones(NSH, np.float32)
        rows_j += [rj[0], rj[1]]
        rows_i += [onesi, onesi]
        rows_j += [ones, ones]
        rows_i += [rj[0][blk], rj[1][blk]]
        mown = np.zeros(N, np.float32)
        mown[blk] = SUP
        rows_j += [mown]
        rows_i += [np.full(NSH, SUP, np.float32)]
        g = np.zeros(N, np.float32)
        h = np.zeros(NSH, np.float32)
        for j in Jstar[core]:
            g[j] = SUP
        for i in Istar[core]:
            h[i] = SUP
        rows_j += [g]
        rows_i += [h]
        featj = np.stack(rows_j).astype(np.float16)
        feati = np.stack(rows_i).astype(np.float16)

        ah = (als / 2.0).astype(np.float32)
        ahj = ah.reshape(64, 128).T.copy()                      # [128, 64]
        ahjo = ah[blk].reshape(8, 128).T.copy()                 # [128, 8]
        ahibc = np.broadcast_to(ah[blk], (128, NSH)).copy()
        cib = np.concatenate(
            [np.broadcast_to(cs[blk, d], (128, NSH)) for d in range(3)],
            axis=1).astype(np.float32).copy()                   # [128, 3072]
        ncjo = np.empty((128, 24), np.float32)
        for t in range(8):
            for d in range(3):
                ncjo[:, t * 3 + d] = -cs[core * NSH + t * 128:
                                         core * NSH + (t + 1) * 128, d]
        in_maps.append({
            "featj": featj, "feati": feati,
            "ahj": np.ascontiguousarray(ahj),
            "ahjo": np.ascontiguousarray(ahjo),
            "ahibc": ahibc, "cib": cib, "ncjo": ncjo,
            "latT": lats[blk].T.astype(np.float16).copy(),
            "w1t": W1.T.astype(np.float16).copy(),
            "w2t": W2.T.astype(np.float16).copy(),
            "b1c": b1.reshape(4, 128).T.astype(np.float32).copy(),
            "b2r": b2.reshape(1, DIM).astype(np.float16),
            "onescol": np.ones((1, 128), np.float16),
            "ones128": np.ones((128, 1), NP_BF16),
        })

    if "nc" not in _BUILT:
        _BUILT["nc"] = build_program()
    nc = _BUILT["nc"]
    res = run_bass_kernel_spmd(nc, in_maps, core_ids=list(range(NCORE)))

    num_all = np.zeros((N, DIM), np.float32)
    s_all = np.zeros(N, np.float32)
    for core in range(NCORE):
        blk = slice(core * NSH, (core + 1) * NSH)
        num_all[blk] = res.results[core]["num"]
        s_all[blk] = res.results[core]["srow"].T.reshape(-1)

    # host fix: add back exact K for suppressed straggler grid J* x I*
    need_rows = sorted(set().union(*Jstar)) if any(Jstar) else []
    if need_rows:
        lr = lats[need_rows]
        hh = lr @ W1.T + b1
        hh = (hh * 0.5 * (1.0 + _erf(hh / np.sqrt(2.0)))).astype(np.float32)
        msg_rows = (hh @ W2.T + b2).astype(np.float32)
        row_pos = {j: k for k, j in enumerate(need_rows)}
        for core in range(NCORE):
            J = sorted(Jstar[core])
            I = sorted(Istar[core])
            if not J or not I:
                continue
            Ig = np.array(I) + core * NSH
            d2c = ((c64[J][:, None, :] - c64[Ig][None, :, :]) ** 2).sum(-1)
            Dc = np.sqrt(d2c)
            aijc = (als[J].astype(np.float64)[:, None]
                    + als[Ig].astype(np.float64)[None, :]) * 0.5
            Kc = (Dc + EPS) ** (-aijc) * np.exp(-Dc / LAM)
            mrows = msg_rows[[row_pos[j] for j in J]]
            num_all[Ig] += (Kc.T @ mrows).astype(np.float32)
            s_all[Ig] += Kc.sum(0).astype(np.float32)

    out = lats + num_all / (s_all[:, None] + 1e-8)
    final = np.empty_like(out)
    final[perm] = out
    return final.astype(np.float32)



# revision 2
# speedup vs baseline: 1.2773x; 1.2773x over previous
"""Trainium2 Bass kernel for ContactDiffusion GNN message passing.

out = latent + K_norm @ msg,  K = (D+eps)^(-alpha_ij) * exp(-D/12), row-normalized,
msg = MLP(latent).

Strategy (8 NeuronCores, SPMD, full inputs in / full output out):
 - Host: KD-sort points spatially; each core owns 1024 contiguous sorted rows.
 - Device per core: pairwise d2 for its [8192 x 1024] K^T slab via a Gram-form
   fp16-split feature matmul (k=18), elementwise K chain fused fp16 on
   ScalarE (single ln/exp activation-table set) + VectorE, contraction +
   row-sums on PE.
 - The core's own diagonal block is computed exactly (ACT Square with
   per-partition bias = direct (ci-cj)^2) with exact ln(D+eps); the Gram pass
   suppresses that block via a rank-1 indicator feature.
 - Cross-core close pairs ("stragglers", d2 < 0.09) are deterministically
   suppressed on device via a second rank-1 indicator feature and their exact
   contribution is added back on host using the exported row sums.
 - MLP is sharded (each core computes msg for its rows); msg is AllGathered.
"""

import math
import os
import sys
from contextlib import ExitStack

import numpy as np

sys.path.insert(0, "/opt/trn_rl_repo")

import ml_dtypes

import concourse.bass as bass
import concourse.tile as tile
from concourse import bacc, mybir
from concourse.bass_utils import run_bass_kernel_spmd

F32 = mybir.dt.float32
F16 = mybir.dt.float16
BF16 = mybir.dt.bfloat16
AF = mybir.ActivationFunctionType
ALU = mybir.AluOpType

NP_BF16 = ml_dtypes.bfloat16

N, DIM, NCORE = 8192, 512, 8
NSH = N // NCORE            # rows per core
EPS, LAM = 1e-4, 12.0
TSTRAG = 0.09               # d2 below this across cores -> straggler
SUP = 1e3                   # suppressor feature magnitude (SUP^2 added to d2)
GROUP = 12                  # j-tiles per psum_out accumulation group
LN12 = math.log(12.0)

_BUILT = {}


# ----------------------------------------------------------------------------
# activation-table pinning: the default chooser maps Ln -> natural_log and
# Exp -> exp_and_others, forcing a ~1.3us ACT_TABLE_LOAD between every pair of
# activations in the K chain.  Restrict the selectable sets (keeping list
# positions, since act_func_set_id is an index into act_info.json) so both
# resolve to natural_log_exp_and_others.
# ----------------------------------------------------------------------------
_ALLOWED_SETS = ("natural_log_exp_and_others", "gelu_and_others")
_gat_patched = False


def _patch_act_tables():
    global _gat_patched
    if _gat_patched:
        return
    import concourse.hw_specs as hw_specs

    orig = hw_specs.get_activation_tables

    def patched(arch):
        tabs = orig(arch)
        return {
            name: (fns if name in _ALLOWED_SETS else set())
            for name, fns in tabs.items()
        }

    bacc.get_activation_tables = patched
    _gat_patched = True


# ----------------------------------------------------------------------------
# device program
# ----------------------------------------------------------------------------
def build_program(n=N, dim=DIM, nsh=NSH, group=GROUP, trace_sim=False, gelu=True):
    nt_own = nsh // 128          # own-block j-tiles
    nt_main = n // 128           # main-pass j-tiles
    n_kd = dim // 128            # contraction k-blocks for MLP
    n_ic = nsh // 128            # i-chunks
    nt_all = nt_own + nt_main

    _patch_act_tables()

    nc = bacc.Bacc("TRN2", target_bir_lowering=False, debug=False,
                   num_devices=NCORE)

    # ---- dram params ----
    featj = nc.dram_tensor("featj", [18, n], F16, kind="ExternalInput").ap()
    feati = nc.dram_tensor("feati", [18, nsh], F16, kind="ExternalInput").ap()
    ahj = nc.dram_tensor("ahj", [128, nt_main], F32, kind="ExternalInput").ap()
    ahjo = nc.dram_tensor("ahjo", [128, nt_own], F32, kind="ExternalInput").ap()
    ahibc = nc.dram_tensor("ahibc", [128, nsh], F16, kind="ExternalInput").ap()
    cib = nc.dram_tensor("cib", [128, 3 * nsh], F32, kind="ExternalInput").ap()
    ncjo = nc.dram_tensor("ncjo", [128, 3 * nt_own], F32, kind="ExternalInput").ap()
    latT = nc.dram_tensor("latT", [dim, nsh], F16, kind="ExternalInput").ap()
    w1t = nc.dram_tensor("w1t", [dim, dim], F16, kind="ExternalInput").ap()
    w2t = nc.dram_tensor("w2t", [dim, dim], F16, kind="ExternalInput").ap()
    b1c = nc.dram_tensor("b1c", [128, n_kd], F32, kind="ExternalInput").ap()
    b2r = nc.dram_tensor("b2r", [1, dim], F16, kind="ExternalInput").ap()
    onescol = nc.dram_tensor("onescol", [1, 128], F16, kind="ExternalInput").ap()
    ones128 = nc.dram_tensor("ones128", [128, 1], BF16, kind="ExternalInput").ap()

    num_out = nc.dram_tensor("num", [nsh, dim], F32, kind="ExternalOutput").ap()
    srow_out = nc.dram_tensor("srow", [128, n_ic], F32, kind="ExternalOutput").ap()

    with tile.TileContext(nc, trace_sim=trace_sim) as tc, ExitStack() as ctx:
        pers = ctx.enter_context(tc.tile_pool(name="pers", bufs=1))
        p_big = ctx.enter_context(tc.tile_pool(name="pbig", bufs=2, space="PSUM"))
        p_out = ctx.enter_context(tc.tile_pool(name="pout", bufs=2, space="PSUM"))
        p_s = ctx.enter_context(tc.tile_pool(name="ps", bufs=1, space="PSUM"))
        sq_pool = ctx.enter_context(tc.tile_pool(name="sq", bufs=1))
        l_pool = ctx.enter_context(tc.tile_pool(name="lp", bufs=2))
        d12_pool = ctx.enter_context(tc.tile_pool(name="d12", bufs=2))
        amt_pool = ctx.enter_context(tc.tile_pool(name="amt", bufs=2))
        k_pool = ctx.enter_context(tc.tile_pool(name="kp", bufs=2 * group + 2))
        kraw_pool = ctx.enter_context(tc.tile_pool(name="kraw", bufs=2))
        msg_pool = ctx.enter_context(tc.tile_pool(name="msgp", bufs=2 * group + 2))
        dram = ctx.enter_context(tc.tile_pool(name="dram", bufs=1, space="DRAM"))

        dma = nc.sync.dma_start

        # ---- persistent SBUF loads ----
        featj_sb = pers.tile([18, n], F16)
        dma(featj_sb[:], featj[:])
        feati_sb = pers.tile([18, nsh], F16)
        dma(feati_sb[:], feati[:])
        ahj_sb = pers.tile([128, nt_main], F32)
        dma(ahj_sb[:], ahj[:])
        ahjo_sb = pers.tile([128, nt_own], F32)
        dma(ahjo_sb[:], ahjo[:])
        ahibc_sb = pers.tile([128, nsh], F16)
        dma(ahibc_sb[:], ahibc[:])
        cib_sb = pers.tile([128, 3 * nsh], F32)
        dma(cib_sb[:], cib[:])
        ncjo_sb = pers.tile([128, 3 * nt_own], F32)
        dma(ncjo_sb[:], ncjo[:])
        b1c_sb = pers.tile([128, n_kd], F32)
        dma(b1c_sb[:], b1c[:])
        b2r_sb = pers.tile([1, dim], F16)
        dma(b2r_sb[:], b2r[:])
        onescol_sb = pers.tile([1, 128], F16)
        dma(onescol_sb[:], onescol[:])
        ones128_sb = pers.tile([128, 1], BF16)
        dma(ones128_sb[:], ones128[:])
        latT_sb = [pers.tile([128, nsh], F16, tag=f"latT{k}", name=f"latT{k}") for k in range(n_kd)]
        for k in range(n_kd):
            dma(latT_sb[k][:], latT[k * 128:(k + 1) * 128, :])
        w1t_sb = [pers.tile([128, dim], F16, tag=f"w1t{k}", name=f"w1t{k}") for k in range(n_kd)]
        w2t_sb = [pers.tile([128, dim], F16, tag=f"w2t{k}", name=f"w2t{k}") for k in range(n_kd)]
        for k in range(n_kd):
            dma(w1t_sb[k][:], w1t[k * 128:(k + 1) * 128, :])
            dma(w2t_sb[k][:], w2t[k * 128:(k + 1) * 128, :])

        acc = pers.tile([128, n_ic * dim], F32)       # out accumulators
        nc.vector.memset(acc[:], 0.0)

        bias_ln12 = pers.tile([128, 1], F32)
        nc.gpsimd.memset(bias_ln12[:], -LN12)
        bias_eps = pers.tile([128, 1], F32)
        nc.gpsimd.memset(bias_eps[:], EPS)
        bias_ln6 = pers.tile([128, 1], F32)
        nc.gpsimd.memset(bias_ln6[:], -math.log(6.0))

        msgown_d = dram.tile([nsh, dim], BF16)
        msgall_d = dram.tile([n, dim], BF16)

        # ---- phase A: MLP (gelu table set) ----
        cw = min(512, nsh)
        hT_sb = [pers.tile([128, nsh], F16, tag=f"hT{k}", name=f"hT{k}") for k in range(n_kd)]
        for mc in range(n_kd):
            ph = p_big.tile([128, nsh], F32, tag="big", name="ph")
            for half in range(nsh // cw):
                hs = slice(half * cw, (half + 1) * cw)
                for kb in range(n_kd):
                    nc.tensor.matmul(
                        ph[:, hs],
                        lhsT=w1t_sb[kb][:, mc * 128:(mc + 1) * 128],
                        rhs=latT_sb[kb][:, hs],
                        start=(kb == 0), stop=(kb == n_kd - 1))
            nc.scalar.activation(hT_sb[mc][:], ph[:], AF.Gelu if gelu else AF.Identity,
                                 bias=b1c_sb[:, mc:mc + 1], scale=1.0)

        msgown_sb = [pers.tile([128, dim], BF16, tag=f"mo{ic}", name=f"mo{ic}") for ic in range(n_ic)]
        for ic in range(n_ic):
            pm = p_out.tile([128, dim], F32, tag="out", name="pm")
            for kb in range(n_kd):
                nc.tensor.matmul(
                    pm[:],
                    lhsT=hT_sb[kb][:, ic * 128:(ic + 1) * 128],
                    rhs=w2t_sb[kb][:],
                    start=(kb == 0), stop=False)
            nc.tensor.matmul(pm[:], lhsT=onescol_sb[:], rhs=b2r_sb[:],
                             start=False, stop=True)
            nc.scalar.copy(msgown_sb[ic][:], pm[:])
            dma(msgown_d[ic * 128:(ic + 1) * 128, :], msgown_sb[ic][:])

        # ---- phase B: AllGather msg ----
        nc.gpsimd.collective_compute(
            "AllGather", ALU.bypass,
            ins=[msgown_d.opt()], outs=[msgall_d.opt()],
            replica_groups=[list(range(NCORE))])

        # ---- phase C/D: slab loop ----
        ps_s = p_s.tile([128, n_ic], F32)

        def emit_elementwise(jt):
            """produce K tile [128, nsh] bf16 + its msg rhs tile; return both"""
            if jt < nt_own:
                # own-block exact pass
                t = jt
                sqs = []
                for d in range(3):
                    sq = sq_pool.tile([128, nsh], F16, tag=f"sq{d}")
                    nc.scalar.activation(
                        sq[:], cib_sb[:, d * nsh:(d + 1) * nsh], AF.Square,
                        bias=ncjo_sb[:, (t * 3 + d):(t * 3 + d) + 1], scale=1.0)
                    sqs.append(sq)
                d2o = sq_pool.tile([128, nsh], F16, tag="d2o")
                nc.vector.tensor_tensor(d2o[:], sqs[0][:], sqs[1][:], op=ALU.add)
                nc.vector.tensor_tensor(d2o[:], d2o[:], sqs[2][:], op=ALU.add)
                l = l_pool.tile([128, nsh], F16)
                nc.scalar.activation(l[:], d2o[:], AF.Ln)
                d12 = d12_pool.tile([128, nsh], F16)
                nc.scalar.activation(d12[:], l[:], AF.Exp, bias=bias_ln12[:, 0:1], scale=0.5)
                bigL = amt_pool.tile([128, nsh], F16, tag="bigL")
                nc.scalar.activation(bigL[:], d12[:], AF.Ln, bias=bias_eps[:, 0:1], scale=12.0)
                al = amt_pool.tile([128, nsh], F16, tag="alpha")
                nc.vector.tensor_scalar_add(al[:], ahibc_sb[:], ahjo_sb[:, t:t + 1])
                m = amt_pool.tile([128, nsh], F16, tag="m")
                nc.vector.tensor_tensor(m[:], al[:], bigL[:], op=ALU.mult)
                tt = amt_pool.tile([128, nsh], F16, tag="t")
                nc.vector.tensor_tensor(tt[:], m[:], d12[:], op=ALU.add)
                kraw = kraw_pool.tile([128, nsh], BF16)
                nc.scalar.activation(kraw[:], tt[:], AF.Exp, scale=-1.0)
                ktile = k_pool.tile([128, nsh], BF16)
                nc.gpsimd.affine_select(
                    ktile[:], kraw[:], pattern=[[1, nsh]],
                    compare_op=ALU.not_equal, fill=0.0,
                    base=-(t * 128), channel_multiplier=-1)
                return ktile, msgown_sb[t]
            # main pass (gram)
            t = jt - nt_own
            pd2 = p_big.tile([128, nsh], F32, tag="big", name="pd2")
            for half in range(nsh // cw):
                hs = slice(half * cw, (half + 1) * cw)
                nc.tensor.matmul(pd2[:, hs],
                                 lhsT=featj_sb[:, t * 128:(t + 1) * 128],
                                 rhs=feati_sb[:, hs],
                                 start=True, stop=True)
            l = l_pool.tile([128, nsh], F16)
            nc.scalar.activation(l[:], pd2[:], AF.Ln)
            d12 = d12_pool.tile([128, nsh], F16)
            nc.scalar.activation(d12[:], l[:], AF.Exp, bias=bias_ln6[:, 0:1], scale=0.5)
            al = amt_pool.tile([128, nsh], F16, tag="alpha")
            nc.vector.tensor_scalar_add(al[:], ahibc_sb[:], ahj_sb[:, t:t + 1])
            m = amt_pool.tile([128, nsh], F16, tag="m")
            nc.vector.tensor_tensor(m[:], al[:], l[:], op=ALU.mult)
            tt = amt_pool.tile([128, nsh], F16, tag="t")
            nc.vector.tensor_tensor(tt[:], m[:], d12[:], op=ALU.add)
            ktile = k_pool.tile([128, nsh], BF16)
            nc.scalar.activation(ktile[:], tt[:], AF.Exp, scale=-0.5)
            mt = msg_pool.tile([128, dim], BF16)
            dma(mt[:], msgall_d[t * 128:(t + 1) * 128, :])
            return ktile, mt

        jt = 0
        while jt < nt_all:
            g = min(group, nt_all - jt)
            tiles = [emit_elementwise(jt + i) for i in range(g)]
            # row-sum matmuls (persistent psum_s accumulation)
            for i, (kt, _) in enumerate(tiles):
                for ic in range(n_ic):
                    nc.tensor.matmul(
                        ps_s[:, ic:ic + 1],
                        lhsT=kt[:, ic * 128:(ic + 1) * 128],
                        rhs=ones128_sb[:],
                        start=(jt + i == 0 and ic == 0),
                        stop=(jt + i == nt_all - 1))
            # contraction for this group
            for ic in range(n_ic):
                po = p_out.tile([128, dim], F32, tag="out", name="po")
                for i, (kt, mt) in enumerate(tiles):
                    nc.tensor.matmul(
                        po[:],
                        lhsT=kt[:, ic * 128:(ic + 1) * 128],
                        rhs=mt[:],
                        start=(i == 0), stop=(i == g - 1))
                asl = slice(ic * dim, (ic + 1) * dim)
                nc.vector.tensor_tensor(acc[:, asl], acc[:, asl], po[:], op=ALU.add)
            jt += g

        # ---- epilogue ----
        ssb = pers.tile([128, n_ic], F32)
        nc.scalar.copy(ssb[:], ps_s[:])
        dma(srow_out[:], ssb[:])
        for ic in range(n_ic):
            dma(num_out[ic * 128:(ic + 1) * 128, :],
                acc[:, ic * dim:(ic + 1) * dim])

    nc.compile()
    return nc


# ----------------------------------------------------------------------------
# host-side preprocessing
# ----------------------------------------------------------------------------
def _kdsort(coords, nblocks):
    def rec(idx, nb):
        if nb == 1:
            return [idx]
        pts = coords[idx]
        ax = int(np.argmax(pts.max(0) - pts.min(0)))
        order = np.argsort(pts[:, ax], kind="stable")
        half = len(idx) // 2
        return rec(idx[order[:half]], nb // 2) + rec(idx[order[half:]], nb // 2)

    return np.concatenate(rec(np.arange(coords.shape[0]), nblocks))


def _split16(x):
    hi = x.astype(np.float16).astype(np.float32)
    lo = (x - hi).astype(np.float16).astype(np.float32)
    return hi, lo


_erf = np.vectorize(math.erf)


def kernel(latent, coords, alpha, W1, b1, W2, b2):
    latent = np.asarray(latent, np.float32)
    coords = np.asarray(coords, np.float32)
    alpha = np.asarray(alpha, np.float32)
    W1 = np.asarray(W1, np.float32)
    b1 = np.asarray(b1, np.float32)
    W2 = np.asarray(W2, np.float32)
    b2 = np.asarray(b2, np.float32)

    perm = _kdsort(coords.astype(np.float64), 64)
    cs = coords[perm]
    als = alpha[perm]
    lats = latent[perm]
    c64 = cs.astype(np.float64)

    core_of = np.arange(N) // NSH
    # stragglers: cross-core pairs with d2 < TSTRAG
    Jstar = [set() for _ in range(NCORE)]
    Istar = [set() for _ in range(NCORE)]
    for i0 in range(0, N, 1024):
        blk = cs[i0:i0 + 1024].astype(np.float64)
        d2b = ((blk[:, None, :] - c64[None, :, :]) ** 2).sum(-1)
        d2b[np.arange(1024), np.arange(i0, i0 + 1024)] = np.inf
        ii, jj = np.nonzero(d2b < TSTRAG)
        ii = ii + i0
        msk = core_of[ii] != core_of[jj]
        for a, b in zip(ii[msk], jj[msk]):
            c = core_of[a]
            Jstar[c].add(int(b))
            Istar[c].add(int(a - c * NSH))

    r = (c64 ** 2).sum(-1).astype(np.float32)
    a2 = (-2.0 * cs).astype(np.float32)
    chj = [_split16(cs[:, d]) for d in range(3)]
    ahi = [_split16(a2[:, d]) for d in range(3)]
    rj = _split16(r)

    in_maps = []
    for core in range(NCORE):
        blk = slice(core * NSH, (core + 1) * NSH)
        rows_j, rows_i = [], []
        for d in range(3):
            for (jp, ip) in [(chj[d][0], ahi[d][0]), (chj[d][0], ahi[d][1]),
                             (chj[d][1], ahi[d][0]), (chj[d][1], ahi[d][1])]:
                rows_j.append(jp)
                rows_i.append(ip[blk])
        ones = np.ones(N, np.float32)
        onesi = np.ones(NSH, np.float32)
        rows_j += [rj[0], rj[1]]
        rows_i += [onesi, onesi]
        rows_j += [ones, ones]
        rows_i += [rj[0][blk], rj[1][blk]]
        mown = np.zeros(N, np.float32)
        mown[blk] = SUP
        rows_j += [mown]
        rows_i += [np.full(NSH, SUP, np.float32)]
        g = np.zeros(N, np.float32)
        h = np.zeros(NSH, np.float32)
        for j in Jstar[core]:
            g[j] = SUP
        for i in Istar[core]:
            h[i] = SUP
        rows_j += [g]
        rows_i += [h]
        featj = np.stack(rows_j).astype(np.float16)
        feati = np.stack(rows_i).astype(np.float16)

        ah = (als / 2.0).astype(np.float32)
        ahj = ah.reshape(64, 128).T.copy()                      # [128, 64]
        ahjo = ah[blk].reshape(8, 128).T.copy()                 # [128, 8]
        ahibc = np.broadcast_to(ah[blk], (128, NSH)).astype(np.float16).copy()
        cib = np.concatenate(
            [np.broadcast_to(cs[blk, d], (128, NSH)) for d in range(3)],
            axis=1).astype(np.float32).copy()                   # [128, 3072]
        ncjo = np.empty((128, 24), np.float32)
        for t in range(8):
            for d in range(3):
                ncjo[:, t * 3 + d] = -cs[core * NSH + t * 128:
                                         core * NSH + (t + 1) * 128, d]
        in_maps.append({
            "featj": featj, "feati": feati,
            "ahj": np.ascontiguousarray(ahj),
            "ahjo": np.ascontiguousarray(ahjo),
            "ahibc": ahibc, "cib": cib, "ncjo": ncjo,
            "latT": lats[blk].T.astype(np.float16).copy(),
            "w1t": W1.T.astype(np.float16).copy(),
            "w2t": W2.T.astype(np.float16).copy(),
            "b1c": b1.reshape(4, 128).T.astype(np.float32).copy(),
            "b2r": b2.reshape(1, DIM).astype(np.float16),
            "onescol": np.ones((1, 128), np.float16),
            "ones128": np.ones((128, 1), NP_BF16),
        })

    if "nc" not in _BUILT:
        _BUILT["nc"] = build_program()
    nc = _BUILT["nc"]
    res = run_bass_kernel_spmd(nc, in_maps, core_ids=list(range(NCORE)))

    num_all = np.zeros((N, DIM), np.float32)
    s_all = np.zeros(N, np.float32)
    for core in range(NCORE):
        blk = slice(core * NSH, (core + 1) * NSH)
        num_all[blk] = res.results[core]["num"]
        s_all[blk] = res.results[core]["srow"].T.reshape(-1)

    # host fix: add back exact K for suppressed straggler grid J* x I*
    need_rows = sorted(set().union(*Jstar)) if any(Jstar) else []
    if need_rows:
        lr = lats[need_rows]
        hh = lr @ W1.T + b1
        hh = (hh * 0.5 * (1.0 + _erf(hh / np.sqrt(2.0)))).astype(np.float32)
        msg_rows = (hh @ W2.T + b2).astype(np.float32)
        row_pos = {j: k for k, j in enumerate(need_rows)}
        for core in range(NCORE):
            J = sorted(Jstar[core])
            I = sorted(Istar[core])
            if not J or not I:
                continue
            Ig = np.array(I) + core * NSH
            d2c = ((c64[J][:, None, :] - c64[Ig][None, :, :]) ** 2).sum(-1)
            Dc = np.sqrt(d2c)
            aijc = (als[J].astype(np.float64)[:, None]
                    + als[Ig].astype(np.float64)[None, :]) * 0.5
            Kc = (Dc + EPS) ** (-aijc) * np.exp(-Dc / LAM)
            mrows = msg_rows[[row_pos[j] for j in J]]
            num_all[Ig] += (Kc.T @ mrows).astype(np.float32)
            s_all[Ig] += Kc.sum(0).astype(np.float32)

    out = lats + num_all / (s_all[:, None] + 1e-8)
    final = np.empty_like(out)
    final[perm] = out
    return final.astype(np.float32)


# revision 3
# speedup vs baseline: 1.4417x; 1.1287x over previous
"""Trainium2 Bass kernel for ContactDiffusion GNN message passing.

out = latent + K_norm @ msg,  K = (D+eps)^(-alpha_ij) * exp(-D/12), row-normalized,
msg = MLP(latent).

Strategy (8 NeuronCores, SPMD, full inputs in / full output out):
 - Host: KD-sort points spatially; each core owns 1024 contiguous sorted rows.
 - Device per core: pairwise d2 for its [8192 x 1024] K^T slab via a Gram-form
   fp16-split feature matmul (k=18), elementwise K chain fp16 on ScalarE
   (single ln/exp activation-table set) + VectorE, contraction on PE.
 - Row-sums via an all-ones stationary matmul accumulated into PSUM.
 - The core's own diagonal block uses per-tile-centered gram features (exact
   to ~1e-6) with an exact ln(D+eps); the main pass suppresses that block via
   a rank-1 indicator feature.
 - Cross-core close pairs ("stragglers", d2 < 0.09) are suppressed on device
   via a second rank-1 indicator feature; exact contribution added on host.
 - MLP is sharded (each core computes msg for its rows); msg is AllGathered.
 - Software-pipelined: group g's contraction matmuls are interleaved into the
   emission of group g+1's elementwise chain so the PE never starves.
"""

import math
import sys
from contextlib import ExitStack

import numpy as np

sys.path.insert(0, "/opt/trn_rl_repo")

import ml_dtypes

import concourse.bass as bass
import concourse.tile as tile
from concourse import bacc, mybir
from concourse.bass_utils import run_bass_kernel_spmd

F32 = mybir.dt.float32
F16 = mybir.dt.float16
BF16 = mybir.dt.bfloat16
AF = mybir.ActivationFunctionType
ALU = mybir.AluOpType

NP_BF16 = ml_dtypes.bfloat16

N, DIM, NCORE = 8192, 512, 8
NSH = N // NCORE            # rows per core
EPS, LAM = 1e-4, 12.0
TSTRAG = 0.09               # d2 below this across cores -> straggler
SUP = 1e3                   # suppressor feature magnitude (SUP^2 added to d2)
GROUP = 12                  # j-tiles per pipelined contraction group
LN12 = math.log(12.0)

_BUILT = {}


# ----------------------------------------------------------------------------
# activation-table pinning: the default chooser maps Ln -> natural_log and
# Exp -> exp_and_others, forcing a ~1.3us ACT_TABLE_LOAD between every pair of
# activations in the K chain.  Restrict the selectable sets (keeping list
# positions, since act_func_set_id is an index into act_info.json) so both
# resolve to natural_log_exp_and_others.
# ----------------------------------------------------------------------------
_ALLOWED_SETS = ("natural_log_exp_and_others", "gelu_and_others")
_gat_patched = False


def _patch_act_tables():
    global _gat_patched
    if _gat_patched:
        return
    import concourse.hw_specs as hw_specs

    orig = hw_specs.get_activation_tables

    def patched(arch):
        tabs = orig(arch)
        return {
            name: (fns if name in _ALLOWED_SETS else set())
            for name, fns in tabs.items()
        }

    bacc.get_activation_tables = patched
    _gat_patched = True


# ----------------------------------------------------------------------------
# device program
# ----------------------------------------------------------------------------
def build_program(n=N, dim=DIM, nsh=NSH, group=GROUP, trace_sim=False, gelu=True):
    nt_own = nsh // 128          # own-block j-tiles
    nt_main = n // 128           # main-pass j-tiles
    n_kd = dim // 128            # contraction k-blocks for MLP
    n_ic = nsh // 128            # i-chunks
    nt_all = nt_own + nt_main

    _patch_act_tables()

    nc = bacc.Bacc("TRN2", target_bir_lowering=False, debug=False,
                   num_devices=NCORE)

    # ---- dram params ----
    featj = nc.dram_tensor("featj", [18, n], F16, kind="ExternalInput").ap()
    feati = nc.dram_tensor("feati", [18, nsh], F16, kind="ExternalInput").ap()
    featjo = nc.dram_tensor("featjo", [16, nsh], F16, kind="ExternalInput").ap()
    featio = nc.dram_tensor("featio", [16, nt_own * nsh], F16, kind="ExternalInput").ap()
    ahj = nc.dram_tensor("ahj", [128, nt_main], F32, kind="ExternalInput").ap()
    ahjo = nc.dram_tensor("ahjo", [128, nt_own], F32, kind="ExternalInput").ap()
    ahibc = nc.dram_tensor("ahibc", [128, nsh], F16, kind="ExternalInput").ap()
    latT = nc.dram_tensor("latT", [dim, nsh], F16, kind="ExternalInput").ap()
    w1t = nc.dram_tensor("w1t", [dim, dim], F16, kind="ExternalInput").ap()
    w2t = nc.dram_tensor("w2t", [dim, dim], F16, kind="ExternalInput").ap()
    b1c = nc.dram_tensor("b1c", [128, n_kd], F32, kind="ExternalInput").ap()
    b2r = nc.dram_tensor("b2r", [1, dim], F16, kind="ExternalInput").ap()
    onescol = nc.dram_tensor("onescol", [1, 128], F16, kind="ExternalInput").ap()
    onesq = nc.dram_tensor("onesq", [128, 128], BF16, kind="ExternalInput").ap()

    num_out = nc.dram_tensor("num", [nsh, dim], F32, kind="ExternalOutput").ap()
    srow_out = nc.dram_tensor("srow", [1, nsh], F32, kind="ExternalOutput").ap()

    with tile.TileContext(nc, trace_sim=trace_sim) as tc, ExitStack() as ctx:
        pers = ctx.enter_context(tc.tile_pool(name="pers", bufs=1))
        p_big = ctx.enter_context(tc.tile_pool(name="pbig", bufs=2, space="PSUM"))
        p_out = ctx.enter_context(tc.tile_pool(name="pout", bufs=2, space="PSUM"))
        p_s = ctx.enter_context(tc.tile_pool(name="ps", bufs=1, space="PSUM"))
        l_pool = ctx.enter_context(tc.tile_pool(name="lp", bufs=2))
        d12_pool = ctx.enter_context(tc.tile_pool(name="d12", bufs=2))
        amt_pool = ctx.enter_context(tc.tile_pool(name="amt", bufs=2))
        k_pool = ctx.enter_context(tc.tile_pool(name="kp", bufs=2 * group + 2))
        kraw_pool = ctx.enter_context(tc.tile_pool(name="kraw", bufs=2))
        msg_pool = ctx.enter_context(tc.tile_pool(name="msgp", bufs=2 * group + 2))
        dram = ctx.enter_context(tc.tile_pool(name="dram", bufs=1, space="DRAM"))

        dma = nc.sync.dma_start

        # ---- persistent SBUF loads ----
        featj_sb = pers.tile([18, n], F16)
        dma(featj_sb[:], featj[:])
        feati_sb = pers.tile([18, nsh], F16)
        dma(feati_sb[:], feati[:])
        featjo_sb = pers.tile([16, nsh], F16)
        dma(featjo_sb[:], featjo[:])
        featio_sb = pers.tile([16, nt_own * nsh], F16)
        dma(featio_sb[:], featio[:])
        ahj_sb = pers.tile([128, nt_main], F32)
        dma(ahj_sb[:], ahj[:])
        ahjo_sb = pers.tile([128, nt_own], F32)
        dma(ahjo_sb[:], ahjo[:])
        ahibc_sb = pers.tile([128, nsh], F16)
        dma(ahibc_sb[:], ahibc[:])
        b1c_sb = pers.tile([128, n_kd], F32)
        dma(b1c_sb[:], b1c[:])
        b2r_sb = pers.tile([1, dim], F16)
        dma(b2r_sb[:], b2r[:])
        onescol_sb = pers.tile([1, 128], F16)
        dma(onescol_sb[:], onescol[:])
        onesq_sb = pers.tile([128, 128], BF16)
        dma(onesq_sb[:], onesq[:])
        latT_sb = [pers.tile([128, nsh], F16, tag=f"latT{k}", name=f"latT{k}") for k in range(n_kd)]
        for k in range(n_kd):
            dma(latT_sb[k][:], latT[k * 128:(k + 1) * 128, :])
        w1t_sb = [pers.tile([128, dim], F16, tag=f"w1t{k}", name=f"w1t{k}") for k in range(n_kd)]
        w2t_sb = [pers.tile([128, dim], F16, tag=f"w2t{k}", name=f"w2t{k}") for k in range(n_kd)]
        for k in range(n_kd):
            dma(w1t_sb[k][:], w1t[k * 128:(k + 1) * 128, :])
            dma(w2t_sb[k][:], w2t[k * 128:(k + 1) * 128, :])

        acc = pers.tile([128, n_ic * dim], F32)       # out accumulators
        nc.vector.memset(acc[:], 0.0)

        bias_ln12 = pers.tile([128, 1], F32)
        nc.gpsimd.memset(bias_ln12[:], -LN12)
        bias_eps = pers.tile([128, 1], F32)
        nc.gpsimd.memset(bias_eps[:], EPS)
        bias_ln6 = pers.tile([128, 1], F32)
        nc.gpsimd.memset(bias_ln6[:], -math.log(6.0))

        msgown_d = dram.tile([nsh, dim], BF16)
        msgall_d = dram.tile([n, dim], BF16)

        # ---- phase A: MLP (gelu table set) ----
        cw = min(512, nsh)
        hT_sb = [pers.tile([128, nsh], F16, tag=f"hT{k}", name=f"hT{k}") for k in range(n_kd)]
        for mc in range(n_kd):
            ph = p_big.tile([128, nsh], F32, tag="big", name="ph")
            for half in range(nsh // cw):
                hs = slice(half * cw, (half + 1) * cw)
                for kb in range(n_kd):
                    nc.tensor.matmul(
                        ph[:, hs],
                        lhsT=w1t_sb[kb][:, mc * 128:(mc + 1) * 128],
                        rhs=latT_sb[kb][:, hs],
                        start=(kb == 0), stop=(kb == n_kd - 1))
            nc.scalar.activation(hT_sb[mc][:], ph[:], AF.Gelu if gelu else AF.Identity,
                                 bias=b1c_sb[:, mc:mc + 1], scale=1.0)

        msgown_sb = [pers.tile([128, dim], BF16, tag=f"mo{ic}", name=f"mo{ic}") for ic in range(n_ic)]
        for ic in range(n_ic):
            pm = p_out.tile([128, dim], F32, tag="out", name="pm")
            for kb in range(n_kd):
                nc.tensor.matmul(
                    pm[:],
                    lhsT=hT_sb[kb][:, ic * 128:(ic + 1) * 128],
                    rhs=w2t_sb[kb][:],
                    start=(kb == 0), stop=False)
            nc.tensor.matmul(pm[:], lhsT=onescol_sb[:], rhs=b2r_sb[:],
                             start=False, stop=True)
            nc.scalar.copy(msgown_sb[ic][:], pm[:])
            dma(msgown_d[ic * 128:(ic + 1) * 128, :], msgown_sb[ic][:])

        # ---- phase B: AllGather msg ----
        nc.gpsimd.collective_compute(
            "AllGather", ALU.bypass,
            ins=[msgown_d.opt()], outs=[msgall_d.opt()],
            replica_groups=[list(range(NCORE))])

        # ---- phase C/D: software-pipelined slab loop ----
        # row-sum accumulator: every partition row ends up holding the same
        # [1, nsh] row-sum vector (ones-stationary matmul trick)
        ps_s = p_s.tile([128, nsh], F32)

        def emit_gram(jt):
            """d2 for tile jt into a PSUM buffer"""
            pd2 = p_big.tile([128, nsh], F32, tag="big", name="pd2")
            if jt < nt_own:
                t = jt
                for half in range(nsh // cw):
                    hs = slice(half * cw, (half + 1) * cw)
                    nc.tensor.matmul(
                        pd2[:, hs],
                        lhsT=featjo_sb[:, t * 128:(t + 1) * 128],
                        rhs=featio_sb[:, t * nsh + half * cw:t * nsh + (half + 1) * cw],
                        start=True, stop=True)
            else:
                t = jt - nt_own
                for half in range(nsh // cw):
                    hs = slice(half * cw, (half + 1) * cw)
                    nc.tensor.matmul(
                        pd2[:, hs],
                        lhsT=featj_sb[:, t * 128:(t + 1) * 128],
                        rhs=feati_sb[:, hs],
                        start=True, stop=True)
            return pd2

        def emit_chain(jt, pd2):
            """elementwise K chain for tile jt; returns (ktile, msg tile)"""
            if jt < nt_own:
                # own-block: exact ln(D+eps) chain
                t = jt
                l = l_pool.tile([128, nsh], F16)
                nc.scalar.activation(l[:], pd2[:], AF.Ln)
                d12 = d12_pool.tile([128, nsh], F16)
                nc.scalar.activation(d12[:], l[:], AF.Exp, bias=bias_ln12[:, 0:1], scale=0.5)
                bigL = amt_pool.tile([128, nsh], F16, tag="bigL")
                nc.scalar.activation(bigL[:], d12[:], AF.Ln, bias=bias_eps[:, 0:1], scale=12.0)
                al = amt_pool.tile([128, nsh], F16, tag="alpha")
                nc.vector.tensor_scalar_add(al[:], ahibc_sb[:], ahjo_sb[:, t:t + 1])
                m = amt_pool.tile([128, nsh], F16, tag="m")
                nc.vector.tensor_tensor(m[:], al[:], bigL[:], op=ALU.mult)
                tt = amt_pool.tile([128, nsh], F16, tag="t")
                nc.vector.tensor_tensor(tt[:], m[:], d12[:], op=ALU.add)
                kraw = kraw_pool.tile([128, nsh], BF16)
                nc.scalar.activation(kraw[:], tt[:], AF.Exp, scale=-1.0)
                ktile = k_pool.tile([128, nsh], BF16)
                nc.gpsimd.affine_select(
                    ktile[:], kraw[:], pattern=[[1, nsh]],
                    compare_op=ALU.not_equal, fill=0.0,
                    base=-(t * 128), channel_multiplier=-1)
                return ktile, msgown_sb[t]
            # main pass
            t = jt - nt_own
            l = l_pool.tile([128, nsh], F16)
            nc.scalar.activation(l[:], pd2[:], AF.Ln)
            d12 = d12_pool.tile([128, nsh], F16)
            nc.scalar.activation(d12[:], l[:], AF.Exp, bias=bias_ln6[:, 0:1], scale=0.5)
            al = amt_pool.tile([128, nsh], F16, tag="alpha")
            nc.vector.tensor_scalar_add(al[:], ahibc_sb[:], ahj_sb[:, t:t + 1])
            m = amt_pool.tile([128, nsh], F16, tag="m")
            nc.vector.tensor_tensor(m[:], al[:], l[:], op=ALU.mult)
            tt = amt_pool.tile([128, nsh], F16, tag="t")
            nc.vector.tensor_tensor(tt[:], m[:], d12[:], op=ALU.add)
            ktile = k_pool.tile([128, nsh], BF16)
            nc.scalar.activation(ktile[:], tt[:], AF.Exp, scale=-0.5)
            mt = msg_pool.tile([128, dim], BF16)
            dma(mt[:], msgall_d[t * 128:(t + 1) * 128, :])
            return ktile, mt

        def contract_stream(tiles, base_jt):
            """flat list of thunks: contraction MMs (ic-major, PSUM-accumulated
            over the group, flushed to acc) interleaved with row-sum MMs"""
            n_t = len(tiles)
            ops = []
            for ic in range(n_ic):
                holder = {}
                for i, (kt, mt) in enumerate(tiles):
                    def mm(ic=ic, i=i, kt=kt, mt=mt, holder=holder, n_t=n_t):
                        if i == 0:
                            holder["po"] = p_out.tile([128, dim], F32, tag="out", name="po")
                        nc.tensor.matmul(
                            holder["po"][:],
                            lhsT=kt[:, ic * 128:(ic + 1) * 128],
                            rhs=mt[:], start=(i == 0), stop=(i == n_t - 1))
                        if i == n_t - 1:
                            asl = slice(ic * dim, (ic + 1) * dim)
                            nc.vector.tensor_tensor(
                                acc[:, asl], acc[:, asl], holder["po"][:], op=ALU.add)
                    ops.append(mm)
            # row-sum MMs: 2 per tile, interleaved evenly (every 5th slot)
            rows = []
            for i, (kt, _) in enumerate(tiles):
                for h in range(nsh // cw):
                    def rmm(i=i, kt=kt, h=h):
                        nc.tensor.matmul(
                            ps_s[:, h * cw:(h + 1) * cw],
                            lhsT=onesq_sb[:],
                            rhs=kt[:, h * cw:(h + 1) * cw],
                            start=(base_jt + i == 0),
                            stop=(base_jt + i == nt_all - 1))
                    rows.append(rmm)
            merged = []
            ri = 0
            for k, op in enumerate(ops):
                merged.append(op)
                if (k + 1) % 4 == 0 and ri < len(rows):
                    merged.append(rows[ri]); ri += 1
            merged.extend(rows[ri:])
            return merged

        prev, prev_base = None, 0
        jt = 0
        while jt < nt_all:
            g = min(group, nt_all - jt)
            grams = [emit_gram(jt + i) for i in range(min(2, g))]
            stream = contract_stream(prev, prev_base) if prev else []
            si = 0
            per_step = (len(stream) + g - 1) // g if stream else 0
            cur = []
            for i in range(g):
                cur.append(emit_chain(jt + i, grams[i]))
                for _ in range(per_step):
                    if si < len(stream):
                        stream[si]()
                        si += 1
                if len(grams) < g:
                    grams.append(emit_gram(jt + len(grams)))
            while si < len(stream):
                stream[si]()
                si += 1
            prev, prev_base = cur, jt
            jt += g
        for op in contract_stream(prev, prev_base):
            op()

        # ---- epilogue ----
        ssb = pers.tile([1, nsh], F32)
        nc.scalar.copy(ssb[:], ps_s[0:1, :])
        dma(srow_out[:], ssb[:])
        for ic in range(n_ic):
            dma(num_out[ic * 128:(ic + 1) * 128, :],
                acc[:, ic * dim:(ic + 1) * dim])

    nc.compile()
    return nc


# ----------------------------------------------------------------------------
# host-side preprocessing
# ----------------------------------------------------------------------------
def _kdsort(coords, nblocks):
    def rec(idx, nb):
        if nb == 1:
            return [idx]
        pts = coords[idx]
        ax = int(np.argmax(pts.max(0) - pts.min(0)))
        order = np.argsort(pts[:, ax], kind="stable")
        half = len(idx) // 2
        return rec(idx[order[:half]], nb // 2) + rec(idx[order[half:]], nb // 2)

    return np.concatenate(rec(np.arange(coords.shape[0]), nblocks))


def _split16(x):
    x = np.asarray(x, np.float32)
    hi = x.astype(np.float16).astype(np.float32)
    lo = (x - hi).astype(np.float16).astype(np.float32)
    return hi, lo


_erf = np.vectorize(math.erf)


def kernel(latent, coords, alpha, W1, b1, W2, b2):
    latent = np.asarray(latent, np.float32)
    coords = np.asarray(coords, np.float32)
    alpha = np.asarray(alpha, np.float32)
    W1 = np.asarray(W1, np.float32)
    b1 = np.asarray(b1, np.float32)
    W2 = np.asarray(W2, np.float32)
    b2 = np.asarray(b2, np.float32)

    perm = _kdsort(coords.astype(np.float64), 64)
    cs = coords[perm]
    als = alpha[perm]
    lats = latent[perm]
    c64 = cs.astype(np.float64)

    core_of = np.arange(N) // NSH
    # stragglers: cross-core pairs with d2 < TSTRAG
    Jstar = [set() for _ in range(NCORE)]
    Istar = [set() for _ in range(NCORE)]
    for i0 in range(0, N, 1024):
        blk = cs[i0:i0 + 1024].astype(np.float64)
        d2b = ((blk[:, None, :] - c64[None, :, :]) ** 2).sum(-1)
        d2b[np.arange(1024), np.arange(i0, i0 + 1024)] = np.inf
        ii, jj = np.nonzero(d2b < TSTRAG)
        ii = ii + i0
        msk = core_of[ii] != core_of[jj]
        for a, b in zip(ii[msk], jj[msk]):
            c = core_of[a]
            Jstar[c].add(int(b))
            Istar[c].add(int(a - c * NSH))

    r = (c64 ** 2).sum(-1).astype(np.float32)
    a2 = (-2.0 * cs).astype(np.float32)
    chj = [_split16(cs[:, d]) for d in range(3)]
    ahi = [_split16(a2[:, d]) for d in range(3)]
    rj = _split16(r)

    in_maps = []
    for core in range(NCORE):
        blk = slice(core * NSH, (core + 1) * NSH)
        rows_j, rows_i = [], []
        for d in range(3):
            for (jp, ip) in [(chj[d][0], ahi[d][0]), (chj[d][0], ahi[d][1]),
                             (chj[d][1], ahi[d][0]), (chj[d][1], ahi[d][1])]:
                rows_j.append(jp)
                rows_i.append(ip[blk])
        ones = np.ones(N, np.float32)
        onesi = np.ones(NSH, np.float32)
        rows_j += [rj[0], rj[1]]
        rows_i += [onesi, onesi]
        rows_j += [ones, ones]
        rows_i += [rj[0][blk], rj[1][blk]]
        mown = np.zeros(N, np.float32)
        mown[blk] = SUP
        rows_j += [mown]
        rows_i += [np.full(NSH, SUP, np.float32)]
        g = np.zeros(N, np.float32)
        h = np.zeros(NSH, np.float32)
        for j in Jstar[core]:
            g[j] = SUP
        for i in Istar[core]:
            h[i] = SUP
        rows_j += [g]
        rows_i += [h]
        featj = np.stack(rows_j).astype(np.float16)
        feati = np.stack(rows_i).astype(np.float16)

        # own-block features: per-tile centered exact gram (16 rows)
        cblk = c64[blk]                                      # [1024, 3]
        featjo = np.zeros((16, NSH), np.float32)
        featio = np.zeros((16, 8 * NSH), np.float32)
        for t in range(8):
            tsl = slice(t * 128, (t + 1) * 128)
            center = cblk[tsl].mean(0)
            cj = (cblk[tsl] - center).astype(np.float32)     # [128, 3]
            ci = (cblk - center).astype(np.float32)          # [1024, 3]
            a2i = -2.0 * ci
            rjq = (cj.astype(np.float64) ** 2).sum(-1).astype(np.float32)
            riq = (ci.astype(np.float64) ** 2).sum(-1).astype(np.float32)
            rj_t, ri_t = _split16(rjq), _split16(riq)
            rrj, rri = [], []
            for d in range(3):
                cjh, cjl = _split16(cj[:, d])
                aih, ail = _split16(a2i[:, d])
                for (jp, ip) in [(cjh, aih), (cjh, ail), (cjl, aih), (cjl, ail)]:
                    rrj.append(jp)
                    rri.append(ip)
            rrj += [rj_t[0], rj_t[1], np.ones(128, np.float32), np.ones(128, np.float32)]
            rri += [np.ones(NSH, np.float32), np.ones(NSH, np.float32), ri_t[0], ri_t[1]]
            featjo[:, tsl] = np.stack(rrj)
            featio[:, t * NSH:(t + 1) * NSH] = np.stack(rri)

        ah = (als / 2.0).astype(np.float32)
        ahj = ah.reshape(64, 128).T.copy()                      # [128, 64]
        ahjo = ah[blk].reshape(8, 128).T.copy()                 # [128, 8]
        ahibc = np.broadcast_to(ah[blk], (128, NSH)).astype(np.float16).copy()
        in_maps.append({
            "featj": featj, "feati": feati,
            "featjo": featjo.astype(np.float16),
            "featio": featio.astype(np.float16),
            "ahj": np.ascontiguousarray(ahj),
            "ahjo": np.ascontiguousarray(ahjo),
            "ahibc": ahibc,
            "latT": lats[blk].T.astype(np.float16).copy(),
            "w1t": W1.T.astype(np.float16).copy(),
            "w2t": W2.T.astype(np.float16).copy(),
            "b1c": b1.reshape(4, 128).T.astype(np.float32).copy(),
            "b2r": b2.reshape(1, DIM).astype(np.float16),
            "onescol": np.ones((1, 128), np.float16),
            "onesq": np.ones((128, 128), NP_BF16),
        })

    if "nc" not in _BUILT:
        _BUILT["nc"] = build_program()
    nc = _BUILT["nc"]
    res = run_bass_kernel_spmd(nc, in_maps, core_ids=list(range(NCORE)))

    num_all = np.zeros((N, DIM), np.float32)
    s_all = np.zeros(N, np.float32)
    for core in range(NCORE):
        blk = slice(core * NSH, (core + 1) * NSH)
        num_all[blk] = res.results[core]["num"]
        s_all[blk] = res.results[core]["srow"].reshape(-1)

    # host fix: add back exact K for suppressed straggler grid J* x I*
    need_rows = sorted(set().union(*Jstar)) if any(Jstar) else []
    if need_rows:
        lr = lats[need_rows]
        hh = lr @ W1.T + b1
        hh = (hh * 0.5 * (1.0 + _erf(hh / np.sqrt(2.0)))).astype(np.float32)
        msg_rows = (hh @ W2.T + b2).astype(np.float32)
        row_pos = {j: k for k, j in enumerate(need_rows)}
        for core in range(NCORE):
            J = sorted(Jstar[core])
            I = sorted(Istar[core])
            if not J or not I:
                continue
            Ig = np.array(I) + core * NSH
            d2c = ((c64[J][:, None, :] - c64[Ig][None, :, :]) ** 2).sum(-1)
            Dc = np.sqrt(d2c)
            aijc = (als[J].astype(np.float64)[:, None]
                    + als[Ig].astype(np.float64)[None, :]) * 0.5
            Kc = (Dc + EPS) ** (-aijc) * np.exp(-Dc / LAM)
            mrows = msg_rows[[row_pos[j] for j in J]]
            num_all[Ig] += (Kc.T @ mrows).astype(np.float32)
            s_all[Ig] += Kc.sum(0).astype(np.float32)

    out = lats + num_all / (s_all[:, None] + 1e-8)
    final = np.empty_like(out)
    final[perm] = out
    return final.astype(np.float32)


# revision 8
# speedup vs baseline: 1.6294x; 1.1302x over previous
"""Trainium2 Bass kernel for ContactDiffusion GNN message passing.

out = latent + K_norm @ msg,  K = (D+eps)^(-alpha_ij) * exp(-D/12), row-normalized,
msg = MLP(latent).

Strategy (8 NeuronCores, SPMD, full inputs in / full output out):
 - Host: KD-sort points spatially; each core owns 1024 contiguous sorted rows.
 - Device per core: pairwise d2 for its [8192 x 1024] K^T slab via a Gram-form
   fp16-split feature matmul (k=18), elementwise K chain fp16 on ScalarE
   (single ln/exp activation-table set) + VectorE, contraction on PE.
 - Row-sums via an all-ones stationary matmul accumulated into PSUM.
 - The core's own diagonal block uses per-tile-centered gram features (exact
   to ~1e-6) with an exact ln(D+eps); the main pass suppresses that block via
   a rank-1 indicator feature.
 - Cross-core close pairs ("stragglers", d2 < 0.09) are suppressed on device
   via a second rank-1 indicator feature; exact contribution added on host.
 - MLP is sharded (each core computes msg for its rows); msg is AllGathered.
 - Software-pipelined: group g's contraction matmuls are interleaved into the
   emission of group g+1's elementwise chain so the PE never starves.
"""

import math
import sys
from contextlib import ExitStack

import numpy as np

sys.path.insert(0, "/opt/trn_rl_repo")

import ml_dtypes

import concourse.bass as bass
import concourse.tile as tile
from concourse import bacc, mybir
from concourse.bass_utils import run_bass_kernel_spmd

F32 = mybir.dt.float32
F16 = mybir.dt.float16
BF16 = mybir.dt.bfloat16
AF = mybir.ActivationFunctionType
ALU = mybir.AluOpType

NP_BF16 = ml_dtypes.bfloat16

N, DIM, NCORE = 8192, 512, 8
NSH = N // NCORE            # rows per core
EPS, LAM = 1e-4, 12.0
TSTRAG = 0.09               # d2 below this across cores -> straggler
SUP = 1e3                   # suppressor feature magnitude (SUP^2 added to d2)
GROUP = 12                  # j-tiles per pipelined contraction group
LN12 = math.log(12.0)

_BUILT = {}


# ----------------------------------------------------------------------------
# activation-table pinning: the default chooser maps Ln -> natural_log and
# Exp -> exp_and_others, forcing a ~1.3us ACT_TABLE_LOAD between every pair of
# activations in the K chain.  Restrict the selectable sets (keeping list
# positions, since act_func_set_id is an index into act_info.json) so both
# resolve to natural_log_exp_and_others.
# ----------------------------------------------------------------------------
_ALLOWED_SETS = ("natural_log_exp_and_others", "gelu_and_others")
_gat_patched = False


def _patch_act_tables():
    global _gat_patched
    if _gat_patched:
        return
    import concourse.hw_specs as hw_specs

    orig = hw_specs.get_activation_tables

    def patched(arch):
        tabs = orig(arch)
        return {
            name: (fns if name in _ALLOWED_SETS else set())
            for name, fns in tabs.items()
        }

    bacc.get_activation_tables = patched
    _gat_patched = True


# ----------------------------------------------------------------------------
# device program
# ----------------------------------------------------------------------------
def build_program(n=N, dim=DIM, nsh=NSH, group=GROUP, trace_sim=False, gelu=True):
    nt_own = nsh // 128          # own-block j-tiles
    nt_main = n // 128           # main-pass j-tiles
    n_kd = dim // 128            # contraction k-blocks for MLP
    n_ic = nsh // 128            # i-chunks
    nt_all = nt_own + nt_main

    _patch_act_tables()

    nc = bacc.Bacc("TRN2", target_bir_lowering=False, debug=False,
                   num_devices=NCORE)

    # ---- dram params ----
    featj = nc.dram_tensor("featj", [18, n], F16, kind="ExternalInput").ap()
    feati = nc.dram_tensor("feati", [18, nsh], F16, kind="ExternalInput").ap()
    featjo = nc.dram_tensor("featjo", [16, nsh], F16, kind="ExternalInput").ap()
    featio = nc.dram_tensor("featio", [16, nt_own * nsh], F16, kind="ExternalInput").ap()
    ahj = nc.dram_tensor("ahj", [128, nt_main], F32, kind="ExternalInput").ap()
    ahjo = nc.dram_tensor("ahjo", [128, nt_own], F32, kind="ExternalInput").ap()
    ahibc = nc.dram_tensor("ahibc", [128, nsh], F16, kind="ExternalInput").ap()
    latT = nc.dram_tensor("latT", [dim, nsh], F16, kind="ExternalInput").ap()
    w1t = nc.dram_tensor("w1t", [dim, dim], F16, kind="ExternalInput").ap()
    w2t = nc.dram_tensor("w2t", [dim, dim], F16, kind="ExternalInput").ap()
    b1c = nc.dram_tensor("b1c", [128, n_kd], F32, kind="ExternalInput").ap()
    b2r = nc.dram_tensor("b2r", [1, dim], F16, kind="ExternalInput").ap()
    onescol = nc.dram_tensor("onescol", [1, 128], F16, kind="ExternalInput").ap()
    onesq = nc.dram_tensor("onesq", [128, 128], BF16, kind="ExternalInput").ap()

    num_out = nc.dram_tensor("num", [nsh, dim], F32, kind="ExternalOutput").ap()
    srow_out = nc.dram_tensor("srow", [1, nsh], F32, kind="ExternalOutput").ap()

    with tile.TileContext(nc, trace_sim=trace_sim) as tc, ExitStack() as ctx:
        pers = ctx.enter_context(tc.tile_pool(name="pers", bufs=1))
        p_big = ctx.enter_context(tc.tile_pool(name="pbig", bufs=2, space="PSUM"))
        p_out = ctx.enter_context(tc.tile_pool(name="pout", bufs=2, space="PSUM"))
        p_s = ctx.enter_context(tc.tile_pool(name="ps", bufs=1, space="PSUM"))
        l_pool = ctx.enter_context(tc.tile_pool(name="lp", bufs=2))
        d12_pool = ctx.enter_context(tc.tile_pool(name="d12", bufs=2))
        amt_pool = ctx.enter_context(tc.tile_pool(name="amt", bufs=2))
        k_pool = ctx.enter_context(tc.tile_pool(name="kp", bufs=2 * group + 2))
        kraw_pool = ctx.enter_context(tc.tile_pool(name="kraw", bufs=2))
        ksum_pool = ctx.enter_context(tc.tile_pool(name="ksum", bufs=3))
        msg_pool = ctx.enter_context(tc.tile_pool(name="msgp", bufs=2 * group + 2))
        dram = ctx.enter_context(tc.tile_pool(name="dram", bufs=1, space="DRAM"))

        dma = nc.sync.dma_start

        # ---- persistent SBUF loads (MLP deps first so phase A starts early) ----
        latT_sb = [pers.tile([128, nsh], F16, tag=f"latT{k}", name=f"latT{k}") for k in range(n_kd)]
        for k in range(n_kd):
            dma(latT_sb[k][:], latT[k * 128:(k + 1) * 128, :])
        w1t_sb = [pers.tile([128, dim], F16, tag=f"w1t{k}", name=f"w1t{k}") for k in range(n_kd)]
        w2t_sb = [pers.tile([128, dim], F16, tag=f"w2t{k}", name=f"w2t{k}") for k in range(n_kd)]
        for k in range(n_kd):
            dma(w1t_sb[k][:], w1t[k * 128:(k + 1) * 128, :])
            dma(w2t_sb[k][:], w2t[k * 128:(k + 1) * 128, :])
        b1c_sb = pers.tile([128, n_kd], F32)
        dma(b1c_sb[:], b1c[:])
        b2r_sb = pers.tile([1, dim], F16)
        dma(b2r_sb[:], b2r[:])
        onescol_sb = pers.tile([1, 128], F16)
        dma(onescol_sb[:], onescol[:])
        # own-block chain deps next, main-pass deps last
        featjo_sb = pers.tile([16, nsh], F16)
        dma(featjo_sb[:], featjo[:])
        featio_sb = pers.tile([16, nt_own * nsh], F16)
        dma(featio_sb[:], featio[:])
        ahjo_sb = pers.tile([128, nt_own], F32)
        dma(ahjo_sb[:], ahjo[:])
        ahibc_sb = pers.tile([128, nsh], F16)
        dma(ahibc_sb[:], ahibc[:])
        featj_sb = pers.tile([18, n], F16)
        dma(featj_sb[:], featj[:])
        feati_sb = pers.tile([18, nsh], F16)
        dma(feati_sb[:], feati[:])
        ahj_sb = pers.tile([128, nt_main], F32)
        dma(ahj_sb[:], ahj[:])
        onesq_sb = pers.tile([128, 128], BF16)
        dma(onesq_sb[:], onesq[:])

        acc = pers.tile([128, n_ic * dim], F32)       # out accumulators
        nc.vector.memset(acc[:], 0.0)

        bias_ln12 = pers.tile([128, 1], F32)
        nc.gpsimd.memset(bias_ln12[:], -LN12)
        bias_eps = pers.tile([128, 1], F32)
        nc.gpsimd.memset(bias_eps[:], EPS)
        bias_ln6 = pers.tile([128, 1], F32)
        nc.gpsimd.memset(bias_ln6[:], -math.log(6.0))

        msgown_d = dram.tile([nsh, dim], BF16)
        msgall_d = dram.tile([n, dim], BF16, addr_space="Shared")

        # ---- phase A: MLP (gelu table set) ----
        cw = min(512, nsh)
        hT_sb = [pers.tile([128, nsh], F16, tag=f"hT{k}", name=f"hT{k}") for k in range(n_kd)]
        for mc in range(n_kd):
            ph = p_big.tile([128, nsh], F32, tag="big", name="ph")
            for half in range(nsh // cw):
                hs = slice(half * cw, (half + 1) * cw)
                for kb in range(n_kd):
                    nc.tensor.matmul(
                        ph[:, hs],
                        lhsT=w1t_sb[kb][:, mc * 128:(mc + 1) * 128],
                        rhs=latT_sb[kb][:, hs],
                        start=(kb == 0), stop=(kb == n_kd - 1))
            nc.scalar.activation(hT_sb[mc][:], ph[:], AF.Gelu if gelu else AF.Identity,
                                 bias=b1c_sb[:, mc:mc + 1], scale=1.0)

        msgown_sb = [pers.tile([128, dim], BF16, tag=f"mo{ic}", name=f"mo{ic}") for ic in range(n_ic)]
        for ic in range(n_ic):
            pm = p_out.tile([128, dim], F32, tag="out", name="pm")
            for kb in range(n_kd):
                nc.tensor.matmul(
                    pm[:],
                    lhsT=hT_sb[kb][:, ic * 128:(ic + 1) * 128],
                    rhs=w2t_sb[kb][:],
                    start=(kb == 0), stop=False)
            nc.tensor.matmul(pm[:], lhsT=onescol_sb[:], rhs=b2r_sb[:],
                             start=False, stop=True)
            nc.scalar.copy(msgown_sb[ic][:], pm[:])
            dma(msgown_d[ic * 128:(ic + 1) * 128, :], msgown_sb[ic][:])

        # ---- phase B: AllGather msg ----
        nc.gpsimd.collective_compute(
            "AllGather", ALU.bypass,
            ins=[msgown_d.opt()], outs=[msgall_d.opt()],
            replica_groups=[list(range(NCORE))])

        # ---- phase C/D: software-pipelined slab loop ----
        # row-sum accumulator: every partition row ends up holding the same
        # [1, nsh] row-sum vector (ones-stationary matmul trick)
        ps_s = p_s.tile([128, nsh], F32)

        def emit_gram(jt):
            """d2 for tile jt into a PSUM buffer"""
            pd2 = p_big.tile([128, nsh], F32, tag="big", name="pd2")
            if jt < nt_own:
                t = jt
                for half in range(nsh // cw):
                    hs = slice(half * cw, (half + 1) * cw)
                    nc.tensor.matmul(
                        pd2[:, hs],
                        lhsT=featjo_sb[:, t * 128:(t + 1) * 128],
                        rhs=featio_sb[:, t * nsh + half * cw:t * nsh + (half + 1) * cw],
                        start=True, stop=True)
            else:
                t = jt - nt_own
                for half in range(nsh // cw):
                    hs = slice(half * cw, (half + 1) * cw)
                    nc.tensor.matmul(
                        pd2[:, hs],
                        lhsT=featj_sb[:, t * 128:(t + 1) * 128],
                        rhs=feati_sb[:, hs],
                        start=True, stop=True)
            return pd2

        def emit_chain(jt, pd2):
            """elementwise K chain for tile jt; returns (ktile, msg tile)"""
            if jt < nt_own:
                # own-block: exact ln(D+eps) chain
                t = jt
                l = l_pool.tile([128, nsh], F16)
                nc.scalar.activation(l[:], pd2[:], AF.Ln)
                d12 = d12_pool.tile([128, nsh], F16)
                nc.scalar.activation(d12[:], l[:], AF.Exp, bias=bias_ln12[:, 0:1], scale=0.5)
                bigL = amt_pool.tile([128, nsh], F16, tag="bigL")
                nc.scalar.activation(bigL[:], d12[:], AF.Ln, bias=bias_eps[:, 0:1], scale=12.0)
                al = amt_pool.tile([128, nsh], F16, tag="alpha")
                nc.vector.tensor_scalar_add(al[:], ahibc_sb[:], ahjo_sb[:, t:t + 1])
                m = amt_pool.tile([128, nsh], F16, tag="m")
                nc.vector.tensor_tensor(m[:], al[:], bigL[:], op=ALU.mult)
                tt = amt_pool.tile([128, nsh], F16, tag="t")
                nc.vector.tensor_tensor(tt[:], m[:], d12[:], op=ALU.add)
                kraw = kraw_pool.tile([128, nsh], BF16)
                nc.scalar.activation(kraw[:], tt[:], AF.Exp, scale=-1.0)
                ktile = k_pool.tile([128, nsh], BF16)
                nc.gpsimd.affine_select(
                    ktile[:], kraw[:], pattern=[[1, nsh]],
                    compare_op=ALU.not_equal, fill=0.0,
                    base=-(t * 128), channel_multiplier=-1)
                return ktile, msgown_sb[t]
            # main pass
            t = jt - nt_own
            l = l_pool.tile([128, nsh], F16)
            nc.scalar.activation(l[:], pd2[:], AF.Ln)
            d12 = d12_pool.tile([128, nsh], F16)
            nc.scalar.activation(d12[:], l[:], AF.Exp, bias=bias_ln6[:, 0:1], scale=0.5)
            al = amt_pool.tile([128, nsh], F16, tag="alpha")
            nc.vector.tensor_scalar_add(al[:], ahibc_sb[:], ahj_sb[:, t:t + 1])
            m = amt_pool.tile([128, nsh], F16, tag="m")
            nc.vector.tensor_tensor(m[:], al[:], l[:], op=ALU.mult)
            tt = amt_pool.tile([128, nsh], F16, tag="t")
            nc.vector.tensor_tensor(tt[:], m[:], d12[:], op=ALU.add)
            ktile = k_pool.tile([128, nsh], BF16)
            nc.scalar.activation(ktile[:], tt[:], AF.Exp, scale=-0.5)
            mt = msg_pool.tile([128, dim], BF16)
            dma(mt[:], msgall_d[t * 128:(t + 1) * 128, :])
            return ktile, mt

        def contract_stream(tiles, first_rs, last_rs):
            """flat list of thunks: contraction MMs (ic-major, PSUM-accumulated
            over the group, flushed to acc) interleaved with paired row-sums
            (kt pairs summed on DVE, halving the row-sum matmul count)"""
            n_t = len(tiles)
            ops = []
            for ic in range(n_ic):
                holder = {}
                for i, (kt, mt) in enumerate(tiles):
                    def mm(ic=ic, i=i, kt=kt, mt=mt, holder=holder, n_t=n_t):
                        if i == 0:
                            holder["po"] = p_out.tile([128, dim], F32, tag="out", name="po")
                        nc.tensor.matmul(
                            holder["po"][:],
                            lhsT=kt[:, ic * 128:(ic + 1) * 128],
                            rhs=mt[:], start=(i == 0), stop=(i == n_t - 1))
                        if i == n_t - 1:
                            asl = slice(ic * dim, (ic + 1) * dim)
                            nc.vector.tensor_tensor(
                                acc[:, asl], acc[:, asl], holder["po"][:], op=ALU.add)
                    ops.append(mm)
            # paired row-sums: sum kt pairs on DVE, then ones-stationary MMs
            rows = []
            pairs = [(tiles[p][0], tiles[p + 1][0] if p + 1 < n_t else None)
                     for p in range(0, n_t, 2)]
            for pi, (ka, kb) in enumerate(pairs):
                holder = {}
                def radd(ka=ka, kb=kb):
                    if kb is None:
                        return ka
                    ks = ksum_pool.tile([128, nsh], BF16)
                    nc.vector.tensor_tensor(ks[:], ka[:], kb[:], op=ALU.add)
                    return ks
                for h in range(nsh // cw):
                    def rmm(radd=radd, h=h, pi=pi, holder=holder, npair=len(pairs)):
                        if h == 0:
                            holder["ks"] = radd()
                        nc.tensor.matmul(
                            ps_s[:, h * cw:(h + 1) * cw],
                            lhsT=onesq_sb[:],
                            rhs=holder["ks"][:, h * cw:(h + 1) * cw],
                            start=(first_rs and pi == 0),
                            stop=(last_rs and pi == npair - 1))
                    rows.append(rmm)
            merged = []
            ri = 0
            for k, op in enumerate(ops):
                merged.append(op)
                if (k + 1) % 8 == 0 and ri < len(rows):
                    merged.append(rows[ri]); ri += 1
            merged.extend(rows[ri:])
            return merged

        prev, prev_last = None, False
        jt = 0
        while jt < nt_all:
            # first group: own tiles only (no msgall dependency) so their
            # contraction overlaps the AllGather latency
            g = nt_own if jt == 0 else min(group, nt_all - jt)
            grams = [emit_gram(jt + i) for i in range(min(2, g))]
            stream = (contract_stream(prev, prev_last, False) if prev else [])
            si = 0
            per_step = (len(stream) + g - 1) // g if stream else 0
            cur = []
            for i in range(g):
                cur.append(emit_chain(jt + i, grams[i]))
                for _ in range(per_step):
                    if si < len(stream):
                        stream[si]()
                        si += 1
                if len(grams) < g:
                    grams.append(emit_gram(jt + len(grams)))
            while si < len(stream):
                stream[si]()
                si += 1
            prev, prev_last = cur, (jt == 0)
            jt += g
        for op in contract_stream(prev, False, True):
            op()

        # ---- epilogue ----
        ssb = pers.tile([1, nsh], F32)
        nc.scalar.copy(ssb[:], ps_s[0:1, :])
        dma(srow_out[:], ssb[:])
        for ic in range(n_ic):
            dma(num_out[ic * 128:(ic + 1) * 128, :],
                acc[:, ic * dim:(ic + 1) * dim])

    nc.compile()
    return nc


# ----------------------------------------------------------------------------
# host-side preprocessing
# ----------------------------------------------------------------------------
def _kdsort(coords, nblocks):
    def rec(idx, nb):
        if nb == 1:
            return [idx]
        pts = coords[idx]
        ax = int(np.argmax(pts.max(0) - pts.min(0)))
        order = np.argsort(pts[:, ax], kind="stable")
        half = len(idx) // 2
        return rec(idx[order[:half]], nb // 2) + rec(idx[order[half:]], nb // 2)

    return np.concatenate(rec(np.arange(coords.shape[0]), nblocks))


def _split16(x):
    x = np.asarray(x, np.float32)
    hi = x.astype(np.float16).astype(np.float32)
    lo = (x - hi).astype(np.float16).astype(np.float32)
    return hi, lo


_erf = np.vectorize(math.erf)


def kernel(latent, coords, alpha, W1, b1, W2, b2):
    latent = np.asarray(latent, np.float32)
    coords = np.asarray(coords, np.float32)
    alpha = np.asarray(alpha, np.float32)
    W1 = np.asarray(W1, np.float32)
    b1 = np.asarray(b1, np.float32)
    W2 = np.asarray(W2, np.float32)
    b2 = np.asarray(b2, np.float32)

    perm = _kdsort(coords.astype(np.float64), 64)
    cs = coords[perm]
    als = alpha[perm]
    lats = latent[perm]
    c64 = cs.astype(np.float64)

    core_of = np.arange(N) // NSH
    # stragglers: cross-core pairs with d2 < TSTRAG
    Jstar = [set() for _ in range(NCORE)]
    Istar = [set() for _ in range(NCORE)]
    for i0 in range(0, N, 1024):
        blk = cs[i0:i0 + 1024].astype(np.float64)
        d2b = ((blk[:, None, :] - c64[None, :, :]) ** 2).sum(-1)
        d2b[np.arange(1024), np.arange(i0, i0 + 1024)] = np.inf
        ii, jj = np.nonzero(d2b < TSTRAG)
        ii = ii + i0
        msk = core_of[ii] != core_of[jj]
        for a, b in zip(ii[msk], jj[msk]):
            c = core_of[a]
            Jstar[c].add(int(b))
            Istar[c].add(int(a - c * NSH))

    r = (c64 ** 2).sum(-1).astype(np.float32)
    a2 = (-2.0 * cs).astype(np.float32)
    chj = [_split16(cs[:, d]) for d in range(3)]
    ahi = [_split16(a2[:, d]) for d in range(3)]
    rj = _split16(r)

    in_maps = []
    for core in range(NCORE):
        blk = slice(core * NSH, (core + 1) * NSH)
        rows_j, rows_i = [], []
        for d in range(3):
            for (jp, ip) in [(chj[d][0], ahi[d][0]), (chj[d][0], ahi[d][1]),
                             (chj[d][1], ahi[d][0]), (chj[d][1], ahi[d][1])]:
                rows_j.append(jp)
                rows_i.append(ip[blk])
        ones = np.ones(N, np.float32)
        onesi = np.ones(NSH, np.float32)
        rows_j += [rj[0], rj[1]]
        rows_i += [onesi, onesi]
        rows_j += [ones, ones]
        rows_i += [rj[0][blk], rj[1][blk]]
        mown = np.zeros(N, np.float32)
        mown[blk] = SUP
        rows_j += [mown]
        rows_i += [np.full(NSH, SUP, np.float32)]
        g = np.zeros(N, np.float32)
        h = np.zeros(NSH, np.float32)
        for j in Jstar[core]:
            g[j] = SUP
        for i in Istar[core]:
            h[i] = SUP
        rows_j += [g]
        rows_i += [h]
        featj = np.stack(rows_j).astype(np.float16)
        feati = np.stack(rows_i).astype(np.float16)

        # own-block features: per-tile centered exact gram (16 rows)
        cblk = c64[blk]                                      # [1024, 3]
        featjo = np.zeros((16, NSH), np.float32)
        featio = np.zeros((16, 8 * NSH), np.float32)
        for t in range(8):
            tsl = slice(t * 128, (t + 1) * 128)
            center = cblk[tsl].mean(0)
            cj = (cblk[tsl] - center).astype(np.float32)     # [128, 3]
            ci = (cblk - center).astype(np.float32)          # [1024, 3]
            a2i = -2.0 * ci
            rjq = (cj.astype(np.float64) ** 2).sum(-1).astype(np.float32)
            riq = (ci.astype(np.float64) ** 2).sum(-1).astype(np.float32)
            rj_t, ri_t = _split16(rjq), _split16(riq)
            rrj, rri = [], []
            for d in range(3):
                cjh, cjl = _split16(cj[:, d])
                aih, ail = _split16(a2i[:, d])
                for (jp, ip) in [(cjh, aih), (cjh, ail), (cjl, aih), (cjl, ail)]:
                    rrj.append(jp)
                    rri.append(ip)
            rrj += [rj_t[0], rj_t[1], np.ones(128, np.float32), np.ones(128, np.float32)]
            rri += [np.ones(NSH, np.float32), np.ones(NSH, np.float32), ri_t[0], ri_t[1]]
            featjo[:, tsl] = np.stack(rrj)
            featio[:, t * NSH:(t + 1) * NSH] = np.stack(rri)

        ah = (als / 2.0).astype(np.float32)
        ahj = ah.reshape(64, 128).T.copy()                      # [128, 64]
        ahjo = ah[blk].reshape(8, 128).T.copy()                 # [128, 8]
        ahibc = np.broadcast_to(ah[blk], (128, NSH)).astype(np.float16).copy()
        in_maps.append({
            "featj": featj, "feati": feati,
            "featjo": featjo.astype(np.float16),
            "featio": featio.astype(np.float16),
            "ahj": np.ascontiguousarray(ahj),
            "ahjo": np.ascontiguousarray(ahjo),
            "ahibc": ahibc,
            "latT": lats[blk].T.astype(np.float16).copy(),
            "w1t": W1.T.astype(np.float16).copy(),
            "w2t": W2.T.astype(np.float16).copy(),
            "b1c": b1.reshape(4, 128).T.astype(np.float32).copy(),
            "b2r": b2.reshape(1, DIM).astype(np.float16),
            "onescol": np.ones((1, 128), np.float16),
            "onesq": np.ones((128, 128), NP_BF16),
        })

    if "nc" not in _BUILT:
        _BUILT["nc"] = build_program()
    nc = _BUILT["nc"]
    res = run_bass_kernel_spmd(nc, in_maps, core_ids=list(range(NCORE)))

    num_all = np.zeros((N, DIM), np.float32)
    s_all = np.zeros(N, np.float32)
    for core in range(NCORE):
        blk = slice(core * NSH, (core + 1) * NSH)
        num_all[blk] = res.results[core]["num"]
        s_all[blk] = res.results[core]["srow"].reshape(-1)

    # host fix: add back exact K for suppressed straggler grid J* x I*
    need_rows = sorted(set().union(*Jstar)) if any(Jstar) else []
    if need_rows:
        lr = lats[need_rows]
        hh = lr @ W1.T + b1
        hh = (hh * 0.5 * (1.0 + _erf(hh / np.sqrt(2.0)))).astype(np.float32)
        msg_rows = (hh @ W2.T + b2).astype(np.float32)
        row_pos = {j: k for k, j in enumerate(need_rows)}
        for core in range(NCORE):
            J = sorted(Jstar[core])
            I = sorted(Istar[core])
            if not J or not I:
                continue
            Ig = np.array(I) + core * NSH
            d2c = ((c64[J][:, None, :] - c64[Ig][None, :, :]) ** 2).sum(-1)
            Dc = np.sqrt(d2c)
            aijc = (als[J].astype(np.float64)[:, None]
                    + als[Ig].astype(np.float64)[None, :]) * 0.5
            Kc = (Dc + EPS) ** (-aijc) * np.exp(-Dc / LAM)
            mrows = msg_rows[[row_pos[j] for j in J]]
            num_all[Ig] += (Kc.T @ mrows).astype(np.float32)
            s_all[Ig] += Kc.sum(0).astype(np.float32)

    out = lats + num_all / (s_all[:, None] + 1e-8)
    final = np.empty_like(out)
    final[perm] = out
    return final.astype(np.float32)
